# revision 16
# baseline (speedup 1.0000x reference)
"""Decoder block on 8 TRN2 NeuronCores — fp16 version.

Sharding: core c -> (batch b=c//2, half h=c%2); each core runs 512 query rows
through the whole decoder with full-T keys/values of its batch element, so no
inter-core communication. Activations feature-major [C, rows]; matmuls are
out[M,N] = lhsT.T @ rhs with fp16 operands and fp32 PSUM accumulation.

Perf structure (vs the fp32r baseline at ~1.4ms):
- fp16 operands: FWL fast weight loads, 2x DVE modes, half DMA traffic.
- Attention S scores for a head pair land in one 2-bank PSUM tile [128,1024]
  so one ACTIVATE does both exps (halves the 352-cycle fixed cost).
- Softmax denominator processing for pair hp is deferred until after the
  next pair's K-production matmuls, so PE never idles on the DVE chain.
- Cross-attention V projection is interleaved into the self-attention loop
  as PE filler (attention alone is ACT-paced); K2 is produced JIT per pair.
- LayerNorm column sums ride along with the projection evictions; the
  mean/rstd broadcasts are staged to fp16 SBUF so normalize runs at 2x DVE.
- FFN is ungrouped: all 32 h tiles materialize, then each output tile
  accumulates the full K=4096 contraction in one PSUM group.
"""

import numpy as np

import concourse.bass as bass
from bass_rust import add_dep_helper
import concourse.mybir as mybir
import concourse.tile as tile
from concourse import bacc
from concourse.bass_utils import run_bass_kernel_spmd

B, T, C, H = 4, 1024, 1024, 16
HD = C // H            # 64
DFF = 4096
EPS = 1e-5
P = 128
R = 512                # query rows per core
FT = C // P            # 8 feature ptiles
RT = T // P            # 8 key-row tiles
NCORES = 8

F32 = mybir.dt.float32
F16 = mybir.dt.float16
AF = mybir.ActivationFunctionType

_CACHE = {}


def _emit(nc, tc, d, flags):
    sync = nc.sync
    ve = nc.vector
    se = nc.scalar
    te = nc.tensor

    # Pin every engine's issue order to emission order (ordering-only deps);
    # prevents the greedy tile scheduler from hoisting an instruction onto a
    # pool slot that only frees much later (queue deadlock).
    _last = {}

    def _chain(key, inst):
        prev = _last.get(key)
        if prev is not None:
            add_dep_helper(inst.ins, prev.ins, sync=False,
                           reason=f"{key} emission-order chain")
        _last[key] = inst
        return inst

    def dma(out, in_):
        # weight/input loads: ordered among themselves so prefetch order is
        # emission order, but NOT behind eviction/output DMAs.
        return _chain("spw", sync.dma_start(out=out, in_=in_))

    def dma_ev(out, in_):
        # SBUF->SBUF shifts and other DVE-fed DMAs
        return _chain("spe", sync.dma_start(out=out, in_=in_))

    def dma_out(out, in_):
        return _chain("spo", sync.dma_start(out=out, in_=in_))

    class _Chained:
        def __init__(self, eng, key):
            self._eng = eng
            self._key = key

        def __getattr__(self, name):
            fn = getattr(self._eng, name)

            def wrapped(*a, **k):
                return _chain(self._key, fn(*a, **k))

            return wrapped

    ve = _Chained(ve, "dve")
    se = _Chained(se, "act")
    te = _Chained(te, "pe")

    pools = {}

    def pool(name, **kw):
        pools[name] = tc.alloc_tile_pool(name=name, **kw)
        return pools[name]

    sb = pool("sb", bufs=1)                      # SBUF, per-tag bufs
    ps_s = pool("ps_s", bufs=2, space="PSUM")    # attention S pairs, 2-bank tiles
    ps_o = pool("ps_o", bufs=2, space="PSUM")    # attention O accum / LN sums
    ps_mm = pool("ps_mm", bufs=2, space="PSUM")  # linear psums / broadcasts

    # ---- constants ----
    ones_t = sb.tile([P, R], F16, tag="ones", bufs=1, name="ones_t")
    ve.memset(ones_t[:], 1.0)
    eps_t = sb.tile([1, 1], F32, tag="eps", bufs=1, name="eps_t")
    ve.memset(eps_t[:], EPS)

    # ---- activation loads (feature-major fp16) ----
    yq_t = []
    for kt in range(FT):
        t = sb.tile([P, R], F16, tag="fm", bufs=34, name=f"yq{kt}")
        dma(out=t[:], in_=d["yq"][kt * P:(kt + 1) * P, :])
        yq_t.append(t)
    ykv_t = []
    for kt in range(FT):
        t = sb.tile([P, T], F16, tag="big", bufs=16, name=f"ykv{kt}")
        dma(out=t[:], in_=d["ykv"][kt * P:(kt + 1) * P, :])
        ykv_t.append(t)
    xkv_t = []
    for kt in range(FT):
        t = sb.tile([P, T], F16, tag="big", bufs=16, name=f"xkv{kt}")
        dma(out=t[:], in_=d["xkv"][kt * P:(kt + 1) * P, :])
        xkv_t.append(t)

    def bias_mm_fm(psum, bias_ap, mt):
        b_t = sb.tile([1, P], F16, tag="bia", bufs=2, name="b_fm")
        dma(out=b_t[:], in_=bias_ap[mt * P:(mt + 1) * P][None, :])
        te.matmul(psum[:], b_t[:], ones_t[0:1, 0:psum.shape[-1]], start=False, stop=True)

    def bias_mm_rm(psum, bias_ap, cc):
        b_t = sb.tile([1, 512], F16, tag="biar", bufs=2, name="b_rm")
        dma(out=b_t[:], in_=bias_ap[cc * 512:(cc + 1) * 512][None, :])
        te.matmul(psum[:], ones_t[0:1, 0:P], b_t[:], start=False, stop=True)

    def linear_fm_mt(in_tiles, w_ap, mt, r, evict, bias_ap=None):
        """One output ptile: out_fm[mt][P, r] = W[:, mt].T @ act."""
        kt_n = len(in_tiles)
        w_t = sb.tile([P, kt_n, P], F16, tag=f"wfm{kt_n}", bufs=3, name="w_fm")
        dma(out=w_t[:],
            in_=w_ap[:, mt * P:(mt + 1) * P].rearrange("(kt p) m -> p kt m", p=P))
        for cc in range(r // 512):
            psm = ps_mm.tile([P, 512], F32, tag="mm", bufs=2, name="ps_lin")
            last = kt_n - 1
            for kt in range(kt_n):
                rhs = in_tiles[kt][:, cc * 512:(cc + 1) * 512]
                te.matmul(psm[:], w_t[:, kt, :], rhs,
                          start=(kt == 0), stop=(kt == last and bias_ap is None))
            if bias_ap is not None:
                bias_mm_fm(psm, bias_ap, mt)
            evict(psm, mt, cc)

    def linear_fm(in_tiles, w_ap, n_out, r, evict, bias_ap=None):
        for mt in range(n_out // P):
            linear_fm_mt(in_tiles, w_ap, mt, r, evict, bias_ap)

    def linear_rm_units(in_tiles, w_ap, n_out, evict, bias_ap=None):
        """Row-major linear as a list of emit-closures (for interleaving):
        unit (cc, rt) computes out[rt][P(rows), 512-chunk cc] = act.T @ W."""
        kt_n = len(in_tiles)
        units = []
        for cc in range(n_out // 512):
            box = {}

            def unit(rt, cc=cc, box=box):
                if rt == 0:
                    w_t = sb.tile([P, kt_n, 512], F16, tag="wrm", bufs=2,
                                  name="w_rm")
                    dma(out=w_t[:],
                        in_=w_ap[:, cc * 512:(cc + 1) * 512]
                        .rearrange("(kt p) m -> p kt m", p=P))
                    box["w"] = w_t
                w_t = box["w"]
                psm = ps_mm.tile([P, 512], F32, tag="mm", bufs=2, name="ps_linr")
                last = kt_n - 1
                for kt in range(kt_n):
                    te.matmul(psm[:], in_tiles[kt][:, rt * P:(rt + 1) * P],
                              w_t[:, kt, :],
                              start=(kt == 0), stop=(kt == last and bias_ap is None))
                if bias_ap is not None:
                    bias_mm_rm(psm, bias_ap, cc)
                evict(psm, rt, cc)

            for rt in range(RT):
                units.append(lambda rt=rt, unit=unit: unit(rt))
        return units

    # ---- layernorm helpers (sums interleave with producer evictions) ----
    lnacc = {}

    def ln_sum_begin(key):
        sx = ps_o.tile([1, 512], F32, tag="o", bufs=2, name=f"{key}sx")
        sx2 = ps_o.tile([1, 512], F32, tag="o", bufs=2, name=f"{key}sx2")
        lnacc[key] = (sx, sx2)

    def ln_sum_step(key, t, idx):
        sx, sx2 = lnacc[key]
        x2 = sb.tile([P, R], F16, tag="x2", bufs=3, name="x2")
        ve.tensor_mul(x2[:], t[:], t[:])
        te.matmul(sx[:], ones_t[:, 0:1], t[:],
                  start=(idx == 0), stop=(idx == FT - 1))
        te.matmul(sx2[:], ones_t[:, 0:1], x2[:],
                  start=(idx == 0), stop=(idx == FT - 1))

    def act_warm(func):
        """Dummy [1,1] activation: preloads the ACT function table while the
        preceding DVE chain runs, so the real activation pays no table load."""
        w = sb.tile([1, 1], F32, tag="warm", bufs=2, name="actwarm")
        se.activation(out=w[:], in_=eps_t[0:1, 0:1], func=func)

    def ln_stats(key):
        """DVE/ACT stat chain -> fp16 [1,512] rstd and mu*rstd."""
        act_warm(AF.Sqrt)
        sx, sx2 = lnacc.pop(key)
        mu = sb.tile([1, 512], F32, tag="stat", bufs=8, name="mu")
        ve.tensor_scalar_mul(mu[:], sx[:], 1.0 / C)
        ex2 = sb.tile([1, 512], F32, tag="stat", bufs=8, name="ex2")
        ve.tensor_scalar_mul(ex2[:], sx2[:], 1.0 / C)
        mu2 = sb.tile([1, 512], F32, tag="stat", bufs=8, name="mu2")
        ve.tensor_mul(mu2[:], mu[:], mu[:])
        var = sb.tile([1, 512], F32, tag="stat", bufs=8, name="var")
        ve.tensor_sub(var[:], ex2[:], mu2[:])
        std = sb.tile([1, 512], F32, tag="stat", bufs=8, name="std")
        se.activation(out=std[:], in_=var[:], func=AF.Sqrt,
                      bias=eps_t[0:1, 0:1], scale=1.0)
        rstd_f = sb.tile([1, 512], F32, tag="stat", bufs=8, name="rstd_f")
        ve.reciprocal_approx_fast(rstd_f[:], std[:])
        rstd = sb.tile([1, 512], F16, tag="stat", bufs=8, name="rstd")
        ve.tensor_copy(rstd[:], rstd_f[:])
        mustd = sb.tile([1, 512], F16, tag="stat", bufs=8, name="mustd")
        ve.tensor_mul(mustd[:], mu[:], rstd_f[:])
        return rstd, mustd

    def ln_bcast(rstd, mustd):
        rbp = ps_mm.tile([P, 512], F32, tag="mm", bufs=2, name="rstd_b")
        te.matmul(rbp[:], ones_t[0:1, 0:P], rstd[:], start=True, stop=True)
        mbp = ps_mm.tile([P, 512], F32, tag="mm", bufs=2, name="mustd_b")
        te.matmul(mbp[:], ones_t[0:1, 0:P], mustd[:], start=True, stop=True)
        rb_s = sb.tile([P, 512], F16, tag="bcs", bufs=4, name="rb_s")
        ve.tensor_copy(rb_s[:], rbp[:])
        mb_s = sb.tile([P, 512], F16, tag="bcs", bufs=4, name="mb_s")
        ve.tensor_copy(mb_s[:], mbp[:])
        return rb_s, mb_s

    def ln_finalize(key):
        return ln_bcast(*ln_stats(key))

    def ln_apply(x_tiles, rb_s, mb_s, out_name, w_ap=None, b_ap=None):
        outs = []
        for kt in range(FT):
            tmp = sb.tile([P, R], F16, tag="lntmp", bufs=2, name="lntmp")
            ve.tensor_mul(tmp[:], x_tiles[kt][:], rb_s[:])
            o = sb.tile([P, R], F16, tag="fm", bufs=34, name=f"{out_name}{kt}")
            if w_ap is None and b_ap is None:
                ve.tensor_sub(o[:], tmp[:], mb_s[:])
            else:
                nrm = sb.tile([P, R], F16, tag="lntmp", bufs=2, name="lnnrm")
                ve.tensor_sub(nrm[:], tmp[:], mb_s[:])
                w_t = sb.tile([P, 1], F32, tag="lnw", bufs=4, name="lnw")
                if w_ap is not None:
                    dma(out=w_t[:], in_=w_ap[kt * P:(kt + 1) * P][:, None])
                else:
                    ve.memset(w_t[:], 1.0)
                b_t = sb.tile([P, 1], F32, tag="lnw", bufs=4, name="lnb")
                if b_ap is not None:
                    dma(out=b_t[:], in_=b_ap[kt * P:(kt + 1) * P][:, None])
                else:
                    ve.memset(b_t[:], 0.0)
                ve.scalar_tensor_tensor(
                    o[:], nrm[:], w_t[:], b_t[:].to_broadcast((P, R)),
                    op0=mybir.AluOpType.mult, op1=mybir.AluOpType.add)
            outs.append(o)
        return outs

    # ---- K producer (JIT per head pair, 2 chunks of 512 keys) ----
    def mk_k_maker(in_tiles, w_ap, bias_ap, pfx):
        def make(hp):
            box = [None]

            def ev(psm, mt, cc):
                if cc == 0:
                    box[0] = sb.tile([P, T], F16, tag="kst", bufs=3,
                                     name=f"{pfx}{hp}")
                ve.tensor_copy(box[0][:, cc * 512:(cc + 1) * 512], psm[:])

            linear_fm_mt(in_tiles, w_ap, hp, T, ev, bias_ap)
            return box[0]
        return make

    # ---- V eviction (row-major with ones column) ----
    def mk_ev_v(v_tiles, tag, pfx):
        def ev(psm, rt, cc):
            if cc == 0:
                v_tiles[rt] = sb.tile([P, H, HD + 1], F16, tag=tag, bufs=8,
                                      name=f"{pfx}{rt}")
            data = ve.tensor_copy(
                v_tiles[rt][:, cc * 8:(cc + 1) * 8, 0:HD],
                psm[:].rearrange("p (h d) -> p h d", d=HD))
            if cc == 0:
                ones_cp = ve.tensor_copy(v_tiles[rt][:, :, HD], ones_t[:, 0:H])
                add_dep_helper(ones_cp.ins, data.ins, sync=False,
                               reason="ones col after first v evict (slot order)")
        return ev

    # ---- attention core (paired exp, pipelined S-ahead, split denom) ----
    def attention(q_tiles, k_maker, v_tiles, o_tiles, pfx,
                  tail_pre=None, tail=None):
        def mk_s(hp, tk, k_hp):
            """S-score pair for (hp, tk) into a fresh 2-bank psum tile."""
            st = ps_s.tile([P, 1024], F32, tag="s", bufs=2, name="st")
            for s in range(2):
                off = HD * s
                te.matmul(st[:, s * 512:(s + 1) * 512],
                          k_hp[off:off + HD, tk * P:(tk + 1) * P],
                          q_tiles[hp][off:off + HD, :],
                          start=True, stop=True)
            return st

        def den_pre(o_ps):
            """DVE reciprocal chain; emitted early so rb never waits."""
            rcrs = []
            for s in range(2):
                den = sb.tile([1, 512], F32, tag="rc", bufs=2, name="den")
                ve.tensor_copy(den[:], o_ps[s][HD:HD + 1, :])
                rc = sb.tile([1, 512], F32, tag="rc", bufs=2, name="rc")
                ve.reciprocal_approx_fast(rc[:], den[:])
                rcr = sb.tile([1, 512], F16, tag="rcr", bufs=2, name="rcr")
                ve.tensor_copy(rcr[:], rc[:])
                rcrs.append(rcr)
            return rcrs

        def den_post(o_ps, o_tile, rcrs, at_tail=False):
            for s in range(2):
                # mid-attention the "mm" slots are safe (k evictions precede
                # the rbs cast on DVE); at the tail the projection psum is
                # held open on "mm", so use the idle S-score slots instead.
                if at_tail:
                    rb = ps_s.tile([HD, 512], F32, tag="s", bufs=2, name="rb")
                else:
                    rb = ps_mm.tile([HD, 512], F32, tag="mm", bufs=2, name="rb")
                te.matmul(rb[:], ones_t[0:1, 0:HD], rcrs[s][:],
                          start=True, stop=True)
                rbs = sb.tile([HD, 512], F16, tag="rbs", bufs=2, name="rbs")
                ve.tensor_copy(rbs[:], rb[:])
                if s == 0:
                    ve.tensor_mul(o_tile[0:HD, :], o_ps[s][0:HD, :], rbs[:])
                else:
                    # DVE cannot shift partitions: normalize at base 0, DMA up.
                    tmp = sb.tile([HD, 512], F16, tag="otmp", bufs=2,
                                  name="otmp")
                    ve.tensor_mul(tmp[:], o_ps[s][0:HD, :], rbs[:])
                    dma_ev(out=o_tile[HD:P, :], in_=tmp[:])

        pend = None  # (o_ps, o_tile, rcrs) awaiting den_post
        for hp in range(H // 2):
            if pend is not None:
                pend = (pend[0], pend[1], den_pre(pend[0]))
            k_hp = k_maker(hp)
            if hp == H // 2 - 1 and tail_pre is not None:
                tail_pre()
            if pend is not None:
                den_post(*pend[0:2], pend[2])
                pend = None
            o_tiles[hp] = sb.tile([P, R], F16, tag="fm", bufs=34,
                                  name=f"{pfx}{hp}")
            o_ps = [ps_o.tile([HD + 1, 512], F32, tag="o", bufs=2,
                              name=f"o_ps{s}") for s in range(2)]
            # software pipeline: S runs one key-tile ahead of exp/O so the
            # PE is never idle during the ACTIVATE.
            sts = {0: mk_s(hp, 0, k_hp)}
            ess = {}
            for tk in range(RT):
                ess[tk] = sb.tile([P, 1024], F16, tag="es", bufs=3, name="es")
                se.activation(out=ess[tk][:], in_=sts[tk][:], func=AF.Exp,
                              scale=0.125)
                if tk + 1 < RT:
                    sts[tk + 1] = mk_s(hp, tk + 1, k_hp)
                for s in range(2):
                    h = 2 * hp + s
                    te.matmul(o_ps[s][:], v_tiles[tk][:, h, :],
                              ess[tk][:, s * 512:(s + 1) * 512],
                              start=(tk == 0), stop=(tk == RT - 1))
                sts.pop(tk)
                ess.pop(tk)
            pend = (o_ps, o_tiles[hp], None)
        # last pair: hide the reciprocal chain behind the caller's tail work
        # (first matmuls of the following projection)
        rcrs = den_pre(pend[0])
        if tail is not None:
            tail()
        den_post(pend[0], pend[1], rcrs, at_tail=True)

    # ================= self-attention qkv =================
    q_t = [None] * FT

    def ev_q(psm, mt, cc):
        t = sb.tile([P, R], F16, tag="fm", bufs=34, name=f"q{mt}")
        ve.tensor_copy(t[:], psm[:])
        q_t[mt] = t

    linear_fm(yq_t, d["W_attn"], C, R, ev_q, bias_ap=d.get("b_attn"))

    k_maker = mk_k_maker(ykv_t, d["W_attn"][:, C:2 * C],
                         (d["b_attn"][C:2 * C] if "b_attn" in d else None), "k")

    v_sb = [None] * RT
    v_units = linear_rm_units(
        ykv_t, d["W_attn"][:, 2 * C:3 * C], C, mk_ev_v(v_sb, "v65", "v"),
        bias_ap=(d["b_attn"][2 * C:3 * C] if "b_attn" in d else None))
    for u in v_units:
        u()

    def proj_tail_hooks(in_tiles, w_ap, bias_ap, ev):
        """mt=0 of a projection, split so its weight load prefetches during
        the last attention pair and its first 7 contraction matmuls hide the
        final denominator chain. finish() does kt=7 + evict + mts 1..7."""
        box = {}

        def tail_pre():
            w_t = sb.tile([P, FT, P], F16, tag="wfm8", bufs=3, name="w_fm")
            dma(out=w_t[:],
                in_=w_ap[:, 0:P].rearrange("(kt p) m -> p kt m", p=P))
            box["w"] = w_t

        def tail():
            psm = ps_mm.tile([P, 512], F32, tag="mm", bufs=2, name="ps_lin")
            for kt in range(FT - 1):
                te.matmul(psm[:], box["w"][:, kt, :], in_tiles[kt][:],
                          start=(kt == 0), stop=False)
            box["psm"] = psm

        def finish():
            psm = box["psm"]
            te.matmul(psm[:], box["w"][:, FT - 1, :], in_tiles[FT - 1][:],
                      start=False, stop=(bias_ap is None))
            if bias_ap is not None:
                bias_mm_fm(psm, bias_ap, 0)
            ev(psm, 0, 0)
            for mt in range(1, FT):
                linear_fm_mt(in_tiles, w_ap, mt, R, ev, bias_ap)

        return tail_pre, tail, finish

    o_all = [None] * FT
    y1 = [None] * FT

    pend_ln = []

    def ev_proj(psm, mt, cc):
        t = sb.tile([P, R], F16, tag="fm", bufs=34, name=f"y1_{mt}")
        ve.tensor_add(t[:], psm[:], yq_t[mt][:])
        y1[mt] = t
        # defer this tile's LN sums by one mt so the PE sum-matmuls never
        # wait on the DVE residual-add + x^2 chain.
        if pend_ln:
            ln_sum_step("ln0", *pend_ln.pop())
        pend_ln.append((t, mt))

    tp0, tl0, fin0 = proj_tail_hooks(o_all, d["W_proj"], d.get("b_proj"),
                                     ev_proj)
    attention(q_t, k_maker, v_sb, o_all, "oall", tail_pre=tp0, tail=tl0)
    ln_sum_begin("ln0")
    fin0()

    # v2 (cross-attention V): emitted here so its slots (shared with v_sb)
    # are free; one unit of PE cover before/after the LN0 stat chain keeps
    # the broadcast matmuls and ln_apply DVE work off the critical path.
    v2_sb = [None] * RT
    v2_units = linear_rm_units(
        xkv_t, d["W_en"][:, C:2 * C], C, mk_ev_v(v2_sb, "v65", "v2_"),
        bias_ap=(d["b_en"][C:2 * C] if "b_en" in d else None))
    for u in v2_units[:1]:
        u()
    ln_sum_step("ln0", *pend_ln.pop())
    st0 = ln_stats("ln0")
    act_warm(AF.Exp)          # table back to Exp before cross-attention
    for u in v2_units[1:2]:
        u()
    rb0, mb0 = ln_bcast(*st0)
    for u in v2_units[2:3]:
        u()
    y1n = ln_apply(y1, rb0, mb0, "y1n",
                   w_ap=d.get("ln_w"), b_ap=d.get("ln_b"))
    for u in v2_units[3:]:
        u()

    # ================= cross attention =================
    k2_maker = mk_k_maker(xkv_t, d["W_en"][:, 0:C],
                          (d["b_en"][0:C] if "b_en" in d else None), "k2_")

    q2_t = [None] * FT

    def ev_q2(psm, mt, cc):
        t = sb.tile([P, R], F16, tag="fm", bufs=34, name=f"q2_{mt}")
        ve.tensor_copy(t[:], psm[:])
        q2_t[mt] = t

    linear_fm(y1n, d["W_q"], C, R, ev_q2, bias_ap=d.get("b_q"))

    o2_all = [None] * FT

    def ev_cproj(psm, mt, cc):
        ve.tensor_add(y1n[mt][:], psm[:], y1n[mt][:])
        if pend_ln:
            ln_sum_step("ln1", *pend_ln.pop())
        pend_ln.append((y1n[mt], mt))

    tp1, tl1, fin1 = proj_tail_hooks(o2_all, d["W_cproj"], d.get("b_cproj"),
                                     ev_cproj)
    attention(q2_t, k2_maker, v2_sb, o2_all, "o2all", tail_pre=tp1, tail=tl1)
    ln_sum_begin("ln1")
    fin1()
    ln_sum_step("ln1", *pend_ln.pop())
    y2 = y1n

    # ================= FFN =================
    rb1, mb1 = ln_finalize("ln1")
    xin = ln_apply(y2, rb1, mb1, "xin", w_ap=d.get("ln1_w"), b_ap=d.get("ln1_b"))

    h_ts = []
    for kt in range(DFF // P):
        w1 = sb.tile([P, FT, P], F16, tag="wfm8", bufs=3, name="w_d1")
        dma(out=w1[:],
            in_=d["W_d1"][:, kt * P:(kt + 1) * P].rearrange("(kt p) m -> p kt m", p=P))
        psm = ps_mm.tile([P, 512], F32, tag="mm", bufs=2, name="ps_h")
        for ck in range(FT):
            te.matmul(psm[:], w1[:, ck, :], xin[ck][:],
                      start=(ck == 0), stop=(ck == FT - 1 and "b_d1" not in d))
        if "b_d1" in d:
            bias_mm_fm(psm, d["b_d1"], kt)
        ht = sb.tile([P, R], F16, tag="ht", bufs=32, name="ht")
        ve.tensor_copy(ht[:], psm[:])
        h_ts.append(ht)

    z_r = [None] * FT
    ln_sum_begin("ln2")
    HK = DFF // P // 2     # 16 k-tiles per weight half
    for mt in range(FT):
        psm = ps_mm.tile([P, 512], F32, tag="mm", bufs=2, name="ps_z")
        for g in range(2):
            w2 = sb.tile([P, HK, P], F16, tag="wd2", bufs=2, name="w_d2")
            dma(out=w2[:],
                in_=d["W_d2"][g * HK * P:(g + 1) * HK * P,
                              mt * P:(mt + 1) * P].rearrange("(kt p) m -> p kt m", p=P))
            for kk in range(HK):
                te.matmul(psm[:], w2[:, kk, :], h_ts[g * HK + kk][:],
                          start=(g == 0 and kk == 0),
                          stop=(g == 1 and kk == HK - 1 and "b_d2" not in d))
        if "b_d2" in d:
            bias_mm_fm(psm, d["b_d2"], mt)
        zr = sb.tile([P, R], F16, tag="fm", bufs=34, name=f"z{mt}")
        ve.tensor_add(zr[:], psm[:], xin[mt][:])
        z_r[mt] = zr
        if pend_ln:
            ln_sum_step("ln2", *pend_ln.pop())
        pend_ln.append((zr, mt))

    ln_sum_step("ln2", *pend_ln.pop())
    rb2, mb2 = ln_finalize("ln2")
    out_tiles = ln_apply(z_r, rb2, mb2, "zo",
                         w_ap=d.get("ln2_w"), b_ap=d.get("ln2_b"))
    for mt in range(FT):
        dma(out=d["out"][mt * P:(mt + 1) * P, :], in_=out_tiles[mt][:])

    for p in reversed(list(pools.values())):
        p.release()


def _build(flags):
    nc = bacc.Bacc(trn_type="TRN2", target_bir_lowering=False, debug=False)
    d = {}

    def din(name, shape, dt=F16):
        d[name] = nc.declare_dram_parameter(name, list(shape), dt, isOutput=False).ap()

    din("yq", (C, R))
    din("ykv", (C, T))
    din("xkv", (C, T))
    din("W_attn", (C, 3 * C))
    din("W_proj", (C, C))
    din("W_en", (C, 2 * C))
    din("W_q", (C, C))
    din("W_cproj", (C, C))
    din("W_d1", (C, DFF))
    din("W_d2", (DFF, C))
    for nm, shape in (("b_attn", (3 * C,)), ("b_proj", (C,)), ("b_en", (2 * C,)),
                      ("b_q", (C,)), ("b_cproj", (C,)), ("b_d1", (DFF,)),
                      ("b_d2", (C,))):
        if nm in flags:
            din(nm, shape)
    for nm in ("ln_w", "ln_b", "ln1_w", "ln1_b", "ln2_w", "ln2_b"):
        if nm in flags:
            din(nm, (C,), dt=F32)
    d["out"] = nc.declare_dram_parameter("out", [C, R], F16, isOutput=True).ap()

    with tile.TileContext(nc) as tc:
        _emit(nc, tc, d, flags)
    nc.compile()
    return nc


def _flags_of(b_attn, b_proj, b_en, b_q, b_cproj, b_d1, b_d2,
              ln_w, ln_b, ln1_w, ln1_b, ln2_w, ln2_b):
    flags = set()
    for nm, arr in (("b_attn", b_attn), ("b_proj", b_proj), ("b_en", b_en),
                    ("b_q", b_q), ("b_cproj", b_cproj), ("b_d1", b_d1),
                    ("b_d2", b_d2)):
        if np.any(np.asarray(arr) != 0):
            flags.add(nm)
    for nm, arr, triv in (("ln_w", ln_w, 1.0), ("ln_b", ln_b, 0.0),
                          ("ln1_w", ln1_w, 1.0), ("ln1_b", ln1_b, 0.0),
                          ("ln2_w", ln2_w, 1.0), ("ln2_b", ln2_b, 0.0)):
        if np.any(np.asarray(arr) != triv):
            flags.add(nm)
    for a, b in (("ln_w", "ln_b"), ("ln1_w", "ln1_b"), ("ln2_w", "ln2_b")):
        if a in flags or b in flags:
            flags.add(a)
            flags.add(b)
    return flags


def prepare_in_maps(x, y, W_attn, b_attn, W_proj, b_proj, ln_w, ln_b,
                    W_en, b_en, W_q, b_q, W_cproj, b_cproj,
                    ln1_w, ln1_b, ln2_w, ln2_b, W_d1, b_d1, W_d2, b_d2,
                    flags):
    f16 = lambda a: np.ascontiguousarray(np.asarray(a, np.float32).astype(np.float16))
    base = {
        "W_attn": f16(W_attn), "W_proj": f16(W_proj), "W_en": f16(W_en),
        "W_q": f16(W_q), "W_cproj": f16(W_cproj), "W_d1": f16(W_d1),
        "W_d2": f16(W_d2),
    }
    opt = {"b_attn": b_attn, "b_proj": b_proj, "b_en": b_en, "b_q": b_q,
           "b_cproj": b_cproj, "b_d1": b_d1, "b_d2": b_d2}
    lnp = {"ln_w": ln_w, "ln_b": ln_b, "ln1_w": ln1_w, "ln1_b": ln1_b,
           "ln2_w": ln2_w, "ln2_b": ln2_b}
    for nm in flags:
        if nm in opt:
            base[nm] = f16(opt[nm])
        else:
            base[nm] = np.ascontiguousarray(lnp[nm], np.float32)

    x = np.asarray(x, np.float32)
    y = np.asarray(y, np.float32)
    yT = [np.ascontiguousarray(y[b].T.astype(np.float16)) for b in range(B)]
    xT = [np.ascontiguousarray(x[b].T.astype(np.float16)) for b in range(B)]
    in_maps = []
    for c in range(NCORES):
        b, h = divmod(c, 2)
        m = dict(base)
        m["ykv"] = yT[b]
        m["xkv"] = xT[b]
        m["yq"] = np.ascontiguousarray(yT[b][:, h * R:(h + 1) * R])
        in_maps.append(m)
    return in_maps


def kernel(x, y, W_attn, b_attn, W_proj, b_proj, ln_w, ln_b,
           W_en, b_en, W_q, b_q, W_cproj, b_cproj,
           ln1_w, ln1_b, ln2_w, ln2_b, W_d1, b_d1, W_d2, b_d2):
    flags = _flags_of(b_attn, b_proj, b_en, b_q, b_cproj, b_d1, b_d2,
                      ln_w, ln_b, ln1_w, ln1_b, ln2_w, ln2_b)
    key = tuple(sorted(flags))
    if key not in _CACHE:
        _CACHE[key] = _build(flags)
    nc = _CACHE[key]

    in_maps = prepare_in_maps(
        x, y, W_attn, b_attn, W_proj, b_proj, ln_w, ln_b,
        W_en, b_en, W_q, b_q, W_cproj, b_cproj,
        ln1_w, ln1_b, ln2_w, ln2_b, W_d1, b_d1, W_d2, b_d2, flags)

    res = run_bass_kernel_spmd(nc, in_maps, list(range(NCORES)))
    out = np.empty((B, T, C), np.float32)
    for c in range(NCORES):
        b, h = divmod(c, 2)
        out[b, h * R:(h + 1) * R, :] = res.results[c]["out"].T.astype(np.float32)
    return out


# revision 21
# speedup vs baseline: 1.0551x; 1.0551x over previous
"""Decoder block on 8 TRN2 NeuronCores — fp16 version.

Sharding: core c -> (batch b=c//2, half h=c%2); each core runs 512 query rows
through the whole decoder with full-T keys/values of its batch element, so no
inter-core communication. Activations feature-major [C, rows]; matmuls are
out[M,N] = lhsT.T @ rhs with fp16 operands and fp32 PSUM accumulation.

Perf structure (vs the fp32r baseline at ~1.4ms):
- fp16 operands: FWL fast weight loads, 2x DVE modes, half DMA traffic.
- Attention S scores for a head pair land in one 2-bank PSUM tile [128,1024]
  so one ACTIVATE does both exps (halves the 352-cycle fixed cost).
- Softmax denominator processing for pair hp is deferred until after the
  next pair's K-production matmuls, so PE never idles on the DVE chain.
- Cross-attention V projection is interleaved into the self-attention loop
  as PE filler (attention alone is ACT-paced); K2 is produced JIT per pair.
- LayerNorm column sums ride along with the projection evictions; the
  mean/rstd broadcasts are staged to fp16 SBUF so normalize runs at 2x DVE.
- FFN is ungrouped: all 32 h tiles materialize, then each output tile
  accumulates the full K=4096 contraction in one PSUM group.
"""

import numpy as np

import concourse.bass as bass
from bass_rust import add_dep_helper
import concourse.mybir as mybir
import concourse.tile as tile
from concourse import bacc
from concourse.bass_utils import run_bass_kernel_spmd

B, T, C, H = 4, 1024, 1024, 16
HD = C // H            # 64
DFF = 4096
EPS = 1e-5
P = 128
R = 512                # query rows per core
FT = C // P            # 8 feature ptiles
RT = T // P            # 8 key-row tiles
NCORES = 8

F32 = mybir.dt.float32
F16 = mybir.dt.float16
AF = mybir.ActivationFunctionType

_CACHE = {}


def _emit(nc, tc, d, flags):
    sync = nc.sync
    ve = nc.vector
    se = nc.scalar
    te = nc.tensor

    # Pin every engine's issue order to emission order (ordering-only deps);
    # prevents the greedy tile scheduler from hoisting an instruction onto a
    # pool slot that only frees much later (queue deadlock).
    _last = {}

    def _chain(key, inst):
        prev = _last.get(key)
        if prev is not None:
            add_dep_helper(inst.ins, prev.ins, sync=False,
                           reason=f"{key} emission-order chain")
        _last[key] = inst
        return inst

    def dma(out, in_):
        # weight/input loads: ordered among themselves so prefetch order is
        # emission order, but NOT behind eviction/output DMAs.
        return _chain("spw", sync.dma_start(out=out, in_=in_))

    def dma_ev(out, in_):
        # SBUF->SBUF shifts and other DVE-fed DMAs
        return _chain("spe", sync.dma_start(out=out, in_=in_))

    def dma_out(out, in_):
        return _chain("spo", sync.dma_start(out=out, in_=in_))

    class _Chained:
        def __init__(self, eng, key):
            self._eng = eng
            self._key = key

        def __getattr__(self, name):
            fn = getattr(self._eng, name)

            def wrapped(*a, **k):
                return _chain(self._key, fn(*a, **k))

            return wrapped

    ve = _Chained(ve, "dve")
    se = _Chained(se, "act")
    te = _Chained(te, "pe")

    pools = {}

    def pool(name, **kw):
        pools[name] = tc.alloc_tile_pool(name=name, **kw)
        return pools[name]

    sb = pool("sb", bufs=1)                      # SBUF, per-tag bufs
    ps_s = pool("ps_s", bufs=2, space="PSUM")    # attention S pairs, 2-bank tiles
    ps_o = pool("ps_o", bufs=2, space="PSUM")    # attention O accum / LN sums
    ps_mm = pool("ps_mm", bufs=2, space="PSUM")  # linear psums / broadcasts

    # ---- constants ----
    ones_t = sb.tile([P, R], F16, tag="ones", bufs=1, name="ones_t")
    ve.memset(ones_t[:], 1.0)
    eps_t = sb.tile([1, 1], F32, tag="eps", bufs=1, name="eps_t")
    ve.memset(eps_t[:], EPS)

    # ---- activation loads (feature-major fp16) ----
    yq_t = []
    for kt in range(FT):
        t = sb.tile([P, R], F16, tag="fm", bufs=34, name=f"yq{kt}")
        dma(out=t[:], in_=d["yq"][kt * P:(kt + 1) * P, :])
        yq_t.append(t)
    ykv_t = []
    for kt in range(FT):
        t = sb.tile([P, T], F16, tag="big", bufs=16, name=f"ykv{kt}")
        dma(out=t[:], in_=d["ykv"][kt * P:(kt + 1) * P, :])
        ykv_t.append(t)
    xkv_t = []
    for kt in range(FT):
        t = sb.tile([P, T], F16, tag="big", bufs=16, name=f"xkv{kt}")
        dma(out=t[:], in_=d["xkv"][kt * P:(kt + 1) * P, :])
        xkv_t.append(t)

    def bias_mm_fm(psum, bias_ap, mt):
        b_t = sb.tile([1, P], F16, tag="bia", bufs=2, name="b_fm")
        dma(out=b_t[:], in_=bias_ap[mt * P:(mt + 1) * P][None, :])
        te.matmul(psum[:], b_t[:], ones_t[0:1, 0:psum.shape[-1]], start=False, stop=True)

    def bias_mm_rm(psum, bias_ap, cc):
        b_t = sb.tile([1, 512], F16, tag="biar", bufs=2, name="b_rm")
        dma(out=b_t[:], in_=bias_ap[cc * 512:(cc + 1) * 512][None, :])
        te.matmul(psum[:], ones_t[0:1, 0:P], b_t[:], start=False, stop=True)

    def linear_fm_mt(in_tiles, w_ap, mt, r, evict, bias_ap=None):
        """One output ptile: out_fm[mt][P, r] = W[:, mt].T @ act.
        w_ap is host-tiled [n_mt, P, kt, P] (contiguous per partition)."""
        kt_n = len(in_tiles)
        w_t = sb.tile([P, kt_n, P], F16, tag=f"wfm{kt_n}", bufs=3, name="w_fm")
        dma(out=w_t[:], in_=w_ap[mt])
        for cc in range(r // 512):
            psm = ps_mm.tile([P, 512], F32, tag="mm", bufs=2, name="ps_lin")
            last = kt_n - 1
            for kt in range(kt_n):
                rhs = in_tiles[kt][:, cc * 512:(cc + 1) * 512]
                te.matmul(psm[:], w_t[:, kt, :], rhs,
                          start=(kt == 0), stop=(kt == last and bias_ap is None))
            if bias_ap is not None:
                bias_mm_fm(psm, bias_ap, mt)
            evict(psm, mt, cc)

    def linear_fm(in_tiles, w_ap, n_out, r, evict, bias_ap=None):
        for mt in range(n_out // P):
            linear_fm_mt(in_tiles, w_ap, mt, r, evict, bias_ap)

    def linear_rm_units(in_tiles, w_ap, n_out, evict, bias_ap=None):
        """Row-major linear as (loads, units) emit-closures for interleaving:
        loads[cc] prefetches that chunk's weights; unit (cc, rt) computes
        out[rt][P(rows), 512-chunk cc] = act.T @ W. w_ap is host-tiled
        [n_cc, P, kt, 512]."""
        kt_n = len(in_tiles)
        units = []
        loads = []
        for cc in range(n_out // 512):
            box = {}

            def load(cc=cc, box=box):
                if "w" not in box:
                    w_t = sb.tile([P, kt_n, 512], F16, tag="wrm", bufs=2,
                                  name="w_rm")
                    dma(out=w_t[:], in_=w_ap[cc])
                    box["w"] = w_t

            loads.append(load)

            def unit(rt, cc=cc, box=box, load=load):
                load()
                w_t = box["w"]
                psm = ps_mm.tile([P, 512], F32, tag="mm", bufs=2, name="ps_linr")
                last = kt_n - 1
                for kt in range(kt_n):
                    te.matmul(psm[:], in_tiles[kt][:, rt * P:(rt + 1) * P],
                              w_t[:, kt, :],
                              start=(kt == 0), stop=(kt == last and bias_ap is None))
                if bias_ap is not None:
                    bias_mm_rm(psm, bias_ap, cc)
                evict(psm, rt, cc)

            for rt in range(RT):
                units.append(lambda rt=rt, unit=unit: unit(rt))
        return loads, units

    # ---- layernorm helpers (sums interleave with producer evictions) ----
    lnacc = {}

    def ln_sum_begin(key):
        sx = ps_o.tile([1, 512], F32, tag="o", bufs=2, name=f"{key}sx")
        sx2 = ps_o.tile([1, 512], F32, tag="o", bufs=2, name=f"{key}sx2")
        lnacc[key] = (sx, sx2)

    def ln_sum_step(key, t, idx):
        sx, sx2 = lnacc[key]
        x2 = sb.tile([P, R], F16, tag="x2", bufs=3, name="x2")
        ve.tensor_mul(x2[:], t[:], t[:])
        te.matmul(sx[:], ones_t[:, 0:1], t[:],
                  start=(idx == 0), stop=(idx == FT - 1))
        te.matmul(sx2[:], ones_t[:, 0:1], x2[:],
                  start=(idx == 0), stop=(idx == FT - 1))

    def act_warm(func):
        """Dummy [1,1] activation: preloads the ACT function table while the
        preceding DVE chain runs, so the real activation pays no table load."""
        w = sb.tile([1, 1], F32, tag="warm", bufs=2, name="actwarm")
        se.activation(out=w[:], in_=eps_t[0:1, 0:1], func=func)

    def ln_stats(key):
        """DVE/ACT stat chain -> fp16 [1,512] rstd and mu*rstd."""
        act_warm(AF.Sqrt)
        sx, sx2 = lnacc.pop(key)
        mu = sb.tile([1, 512], F32, tag="stat", bufs=8, name="mu")
        ve.tensor_scalar_mul(mu[:], sx[:], 1.0 / C)
        ex2 = sb.tile([1, 512], F32, tag="stat", bufs=8, name="ex2")
        ve.tensor_scalar_mul(ex2[:], sx2[:], 1.0 / C)
        mu2 = sb.tile([1, 512], F32, tag="stat", bufs=8, name="mu2")
        ve.tensor_mul(mu2[:], mu[:], mu[:])
        var = sb.tile([1, 512], F32, tag="stat", bufs=8, name="var")
        ve.tensor_sub(var[:], ex2[:], mu2[:])
        std = sb.tile([1, 512], F32, tag="stat", bufs=8, name="std")
        se.activation(out=std[:], in_=var[:], func=AF.Sqrt,
                      bias=eps_t[0:1, 0:1], scale=1.0)
        rstd_f = sb.tile([1, 512], F32, tag="stat", bufs=8, name="rstd_f")
        ve.reciprocal_approx_fast(rstd_f[:], std[:])
        rstd = sb.tile([1, 512], F16, tag="stat", bufs=8, name="rstd")
        ve.tensor_copy(rstd[:], rstd_f[:])
        mustd = sb.tile([1, 512], F16, tag="stat", bufs=8, name="mustd")
        ve.tensor_mul(mustd[:], mu[:], rstd_f[:])
        return rstd, mustd

    def ln_bcast(rstd, mustd):
        rbp = ps_mm.tile([P, 512], F32, tag="mm", bufs=2, name="rstd_b")
        te.matmul(rbp[:], ones_t[0:1, 0:P], rstd[:], start=True, stop=True)
        mbp = ps_mm.tile([P, 512], F32, tag="mm", bufs=2, name="mustd_b")
        te.matmul(mbp[:], ones_t[0:1, 0:P], mustd[:], start=True, stop=True)
        rb_s = sb.tile([P, 512], F16, tag="bcs", bufs=4, name="rb_s")
        ve.tensor_copy(rb_s[:], rbp[:])
        mb_s = sb.tile([P, 512], F16, tag="bcs", bufs=4, name="mb_s")
        ve.tensor_copy(mb_s[:], mbp[:])
        return rb_s, mb_s

    def ln_finalize(key):
        return ln_bcast(*ln_stats(key))

    def ln_apply(x_tiles, rb_s, mb_s, out_name, w_ap=None, b_ap=None):
        outs = []
        for kt in range(FT):
            tmp = sb.tile([P, R], F16, tag="lntmp", bufs=2, name="lntmp")
            ve.tensor_mul(tmp[:], x_tiles[kt][:], rb_s[:])
            o = sb.tile([P, R], F16, tag="fm", bufs=34, name=f"{out_name}{kt}")
            if w_ap is None and b_ap is None:
                ve.tensor_sub(o[:], tmp[:], mb_s[:])
            else:
                nrm = sb.tile([P, R], F16, tag="lntmp", bufs=2, name="lnnrm")
                ve.tensor_sub(nrm[:], tmp[:], mb_s[:])
                w_t = sb.tile([P, 1], F32, tag="lnw", bufs=4, name="lnw")
                if w_ap is not None:
                    dma(out=w_t[:], in_=w_ap[kt * P:(kt + 1) * P][:, None])
                else:
                    ve.memset(w_t[:], 1.0)
                b_t = sb.tile([P, 1], F32, tag="lnw", bufs=4, name="lnb")
                if b_ap is not None:
                    dma(out=b_t[:], in_=b_ap[kt * P:(kt + 1) * P][:, None])
                else:
                    ve.memset(b_t[:], 0.0)
                ve.scalar_tensor_tensor(
                    o[:], nrm[:], w_t[:], b_t[:].to_broadcast((P, R)),
                    op0=mybir.AluOpType.mult, op1=mybir.AluOpType.add)
            outs.append(o)
        return outs

    # ---- K producer (JIT per head pair, 2 chunks of 512 keys) ----
    def mk_k_maker(in_tiles, w_ap, bias_ap, pfx):
        def make(hp):
            box = [None]

            def ev(psm, mt, cc):
                if cc == 0:
                    box[0] = sb.tile([P, T], F16, tag="kst", bufs=3,
                                     name=f"{pfx}{hp}")
                ve.tensor_copy(box[0][:, cc * 512:(cc + 1) * 512], psm[:])

            linear_fm_mt(in_tiles, w_ap, hp, T, ev, bias_ap)
            return box[0]
        return make

    # ---- V eviction (row-major with ones column) ----
    def mk_ev_v(v_tiles, tag, pfx):
        def ev(psm, rt, cc):
            if cc == 0:
                v_tiles[rt] = sb.tile([P, H, HD + 1], F16, tag=tag, bufs=8,
                                      name=f"{pfx}{rt}")
            data = ve.tensor_copy(
                v_tiles[rt][:, cc * 8:(cc + 1) * 8, 0:HD],
                psm[:].rearrange("p (h d) -> p h d", d=HD))
            if cc == 0:
                ones_cp = ve.tensor_copy(v_tiles[rt][:, :, HD], ones_t[:, 0:H])
                add_dep_helper(ones_cp.ins, data.ins, sync=False,
                               reason="ones col after first v evict (slot order)")
        return ev

    # ---- attention core (paired exp, pipelined S-ahead, split denom) ----
    def attention(q_tiles, k_maker, v_tiles, o_tiles, pfx,
                  tail_pre=None, tail=None):
        def mk_s(hp, tk, k_hp):
            """S-score pair for (hp, tk) into a fresh 2-bank psum tile."""
            st = ps_s.tile([P, 1024], F32, tag="s", bufs=2, name="st")
            for s in range(2):
                off = HD * s
                te.matmul(st[:, s * 512:(s + 1) * 512],
                          k_hp[off:off + HD, tk * P:(tk + 1) * P],
                          q_tiles[hp][off:off + HD, :],
                          start=True, stop=True)
            return st

        def den_pre(o_ps):
            """DVE reciprocal chain; emitted early so rb never waits."""
            rcrs = {}
            for s in (1, 0):
                den = sb.tile([1, 512], F32, tag="rc", bufs=2, name="den")
                ve.tensor_copy(den[:], o_ps[s][HD:HD + 1, :])
                rc = sb.tile([1, 512], F32, tag="rc", bufs=2, name="rc")
                ve.reciprocal_approx_fast(rc[:], den[:])
                rcr = sb.tile([1, 512], F16, tag="rcr", bufs=2, name="rcr")
                ve.tensor_copy(rcr[:], rc[:])
                rcrs[s] = rcr
            return rcrs

        def den_post(o_ps, o_tile, rcrs, at_tail=False):
            for s in (1, 0):
                # mid-attention the "mm" slots are safe (k evictions precede
                # the rbs cast on DVE); at the tail the projection psum is
                # held open on "mm", so use the idle S-score slots instead.
                if at_tail:
                    rb = ps_s.tile([HD, 512], F32, tag="s", bufs=2, name="rb")
                else:
                    rb = ps_mm.tile([HD, 512], F32, tag="mm", bufs=2, name="rb")
                te.matmul(rb[:], ones_t[0:1, 0:HD], rcrs[s][:],
                          start=True, stop=True)
                rbs = sb.tile([HD, 512], F16, tag="rbs", bufs=2, name="rbs")
                ve.tensor_copy(rbs[:], rb[:])
                if s == 0:
                    ve.tensor_mul(o_tile[0:HD, :], o_ps[s][0:HD, :], rbs[:])
                else:
                    # DVE cannot shift partitions: normalize at base 0, DMA up.
                    tmp = sb.tile([HD, 512], F16, tag="otmp", bufs=2,
                                  name="otmp")
                    ve.tensor_mul(tmp[:], o_ps[s][0:HD, :], rbs[:])
                    dma_ev(out=o_tile[HD:P, :], in_=tmp[:])

        pend = None  # (o_ps, o_tile, rcrs) awaiting den_post
        for hp in range(H // 2):
            if pend is not None:
                pend = (pend[0], pend[1], den_pre(pend[0]))
            k_hp = k_maker(hp)
            if hp == H // 2 - 1 and tail_pre is not None:
                tail_pre()
            if pend is not None:
                den_post(*pend[0:2], pend[2])
                pend = None
            o_tiles[hp] = sb.tile([P, R], F16, tag="fm", bufs=34,
                                  name=f"{pfx}{hp}")
            o_ps = [ps_o.tile([HD + 1, 512], F32, tag="o", bufs=2,
                              name=f"o_ps{s}") for s in range(2)]
            # software pipeline: S runs one key-tile ahead of exp/O so the
            # PE is never idle during the ACTIVATE.
            sts = {0: mk_s(hp, 0, k_hp)}
            ess = {}
            for tk in range(RT):
                ess[tk] = sb.tile([P, 1024], F16, tag="es", bufs=3, name="es")
                se.activation(out=ess[tk][:], in_=sts[tk][:], func=AF.Exp,
                              scale=0.125)
                if tk + 1 < RT:
                    sts[tk + 1] = mk_s(hp, tk + 1, k_hp)
                for s in range(2):
                    h = 2 * hp + s
                    te.matmul(o_ps[s][:], v_tiles[tk][:, h, :],
                              ess[tk][:, s * 512:(s + 1) * 512],
                              start=(tk == 0), stop=(tk == RT - 1))
                sts.pop(tk)
                ess.pop(tk)
            pend = (o_ps, o_tiles[hp], None)
        # last pair: hide the reciprocal chain behind the caller's tail work
        # (first matmuls of the following projection)
        rcrs = den_pre(pend[0])
        if tail is not None:
            tail()
        den_post(pend[0], pend[1], rcrs, at_tail=True)

    # ================= self-attention qkv =================
    q_t = [None] * FT

    def ev_q(psm, mt, cc):
        t = sb.tile([P, R], F16, tag="fm", bufs=34, name=f"q{mt}")
        ve.tensor_copy(t[:], psm[:])
        q_t[mt] = t

    linear_fm(yq_t, d["Wq_t"], C, R, ev_q, bias_ap=d.get("b_attn"))

    k_maker = mk_k_maker(ykv_t, d["Wk_t"],
                         (d["b_attn"][C:2 * C] if "b_attn" in d else None), "k")

    v_sb = [None] * RT
    v_loads, v_units = linear_rm_units(
        ykv_t, d["Wv_t"], C, mk_ev_v(v_sb, "v65", "v"),
        bias_ap=(d["b_attn"][2 * C:3 * C] if "b_attn" in d else None))
    for u in v_units:
        u()

    o_all = [None] * FT
    y1 = [None] * FT

    pend_ln = []

    def ev_proj(psm, mt, cc):
        # flush the previous tile's LN sums FIRST: its x^2 then runs on DVE
        # ahead of this tile's residual add (whose psum isn't ready yet).
        if pend_ln:
            ln_sum_step("ln0", *pend_ln.pop())
        t = sb.tile([P, R], F16, tag="fm", bufs=34, name=f"y1_{mt}")
        ve.tensor_add(t[:], psm[:], yq_t[mt][:])
        y1[mt] = t
        pend_ln.append((t, mt))

    # v2 (cross-attention V): its first two units serve as the self-attention
    # tail — independent PE work that hides the last pair's denominator chain
    # (reciprocals, broadcasts, normalize muls, partition-shift DMA), so the
    # projection afterwards never waits on o_all[7].
    v2_sb = [None] * RT
    v2_loads, v2_units = linear_rm_units(
        xkv_t, d["Wen_v_t"], C, mk_ev_v(v2_sb, "v65", "v2_"),
        bias_ap=(d["b_en"][C:2 * C] if "b_en" in d else None))

    def tl0():
        v2_units[0]()
        v2_units[1]()

    attention(q_t, k_maker, v_sb, o_all, "oall",
              tail_pre=v2_loads[0], tail=tl0)
    ln_sum_begin("ln0")
    linear_fm(o_all, d["Wproj_t"], C, R, ev_proj, bias_ap=d.get("b_proj"))

    for u in v2_units[2:3]:
        u()
    ln_sum_step("ln0", *pend_ln.pop())
    st0 = ln_stats("ln0")
    act_warm(AF.Exp)          # table back to Exp before cross-attention
    for u in v2_units[3:4]:
        u()
    rb0, mb0 = ln_bcast(*st0)
    for u in v2_units[4:5]:
        u()
    y1n = ln_apply(y1, rb0, mb0, "y1n",
                   w_ap=d.get("ln_w"), b_ap=d.get("ln_b"))
    for u in v2_units[5:]:
        u()

    # ================= cross attention =================
    k2_maker = mk_k_maker(xkv_t, d["Wen_k_t"],
                          (d["b_en"][0:C] if "b_en" in d else None), "k2_")

    q2_t = [None] * FT

    def ev_q2(psm, mt, cc):
        t = sb.tile([P, R], F16, tag="fm", bufs=34, name=f"q2_{mt}")
        ve.tensor_copy(t[:], psm[:])
        q2_t[mt] = t

    linear_fm(y1n, d["Wq2_t"], C, R, ev_q2, bias_ap=d.get("b_q"))

    o2_all = [None] * FT

    def ev_cproj(psm, mt, cc):
        if pend_ln:
            ln_sum_step("ln1", *pend_ln.pop())
        ve.tensor_add(y1n[mt][:], psm[:], y1n[mt][:])
        pend_ln.append((y1n[mt], mt))

    # cproj mt=0 split: weight prefetched during the last cross pair, its
    # first 7 contraction matmuls hide the final denominator chain.
    cpbox = {}

    def tp1():
        w_t = sb.tile([P, FT, P], F16, tag="wfm8", bufs=3, name="w_fm")
        dma(out=w_t[:], in_=d["Wcproj_t"][0])
        cpbox["w"] = w_t

    def tl1():
        psm = ps_mm.tile([P, 512], F32, tag="mm", bufs=2, name="ps_lin")
        for kt in range(FT - 1):
            te.matmul(psm[:], cpbox["w"][:, kt, :], o2_all[kt][:],
                      start=(kt == 0), stop=False)
        cpbox["psm"] = psm

    attention(q2_t, k2_maker, v2_sb, o2_all, "o2all", tail_pre=tp1, tail=tl1)
    ln_sum_begin("ln1")
    b_cproj = d.get("b_cproj")
    psm = cpbox["psm"]
    te.matmul(psm[:], cpbox["w"][:, FT - 1, :], o2_all[FT - 1][:],
              start=False, stop=(b_cproj is None))
    if b_cproj is not None:
        bias_mm_fm(psm, b_cproj, 0)
    ev_cproj(psm, 0, 0)
    for mt in range(1, FT):
        linear_fm_mt(o2_all, d["Wcproj_t"], mt, R, ev_cproj, b_cproj)
    ln_sum_step("ln1", *pend_ln.pop())
    y2 = y1n

    # ================= FFN =================
    rb1, mb1 = ln_finalize("ln1")
    xin = ln_apply(y2, rb1, mb1, "xin", w_ap=d.get("ln1_w"), b_ap=d.get("ln1_b"))

    h_ts = []
    for kt in range(DFF // P):
        w1 = sb.tile([P, FT, P], F16, tag="wfm8", bufs=3, name="w_d1")
        dma(out=w1[:], in_=d["Wd1_t"][kt])
        psm = ps_mm.tile([P, 512], F32, tag="mm", bufs=2, name="ps_h")
        for ck in range(FT):
            te.matmul(psm[:], w1[:, ck, :], xin[ck][:],
                      start=(ck == 0), stop=(ck == FT - 1 and "b_d1" not in d))
        if "b_d1" in d:
            bias_mm_fm(psm, d["b_d1"], kt)
        ht = sb.tile([P, R], F16, tag="ht", bufs=32, name="ht")
        ve.tensor_copy(ht[:], psm[:])
        h_ts.append(ht)

    z_r = [None] * FT
    ln_sum_begin("ln2")
    HK = DFF // P // 2     # 16 k-tiles per weight half
    for mt in range(FT):
        psm = ps_mm.tile([P, 512], F32, tag="mm", bufs=2, name="ps_z")
        for g in range(2):
            w2 = sb.tile([P, HK, P], F16, tag="wd2", bufs=2, name="w_d2")
            dma(out=w2[:], in_=d["Wd2_t"][mt][:, g * HK:(g + 1) * HK, :])
            for kk in range(HK):
                te.matmul(psm[:], w2[:, kk, :], h_ts[g * HK + kk][:],
                          start=(g == 0 and kk == 0),
                          stop=(g == 1 and kk == HK - 1 and "b_d2" not in d))
        if "b_d2" in d:
            bias_mm_fm(psm, d["b_d2"], mt)
        if pend_ln:
            ln_sum_step("ln2", *pend_ln.pop())
        zr = sb.tile([P, R], F16, tag="fm", bufs=34, name=f"z{mt}")
        ve.tensor_add(zr[:], psm[:], xin[mt][:])
        z_r[mt] = zr
        pend_ln.append((zr, mt))

    ln_sum_step("ln2", *pend_ln.pop())
    rb2, mb2 = ln_finalize("ln2")
    out_tiles = ln_apply(z_r, rb2, mb2, "zo",
                         w_ap=d.get("ln2_w"), b_ap=d.get("ln2_b"))
    for mt in range(FT):
        dma(out=d["out"][mt * P:(mt + 1) * P, :], in_=out_tiles[mt][:])

    for p in reversed(list(pools.values())):
        p.release()


def _build(flags):
    nc = bacc.Bacc(trn_type="TRN2", target_bir_lowering=False, debug=False)
    d = {}

    def din(name, shape, dt=F16):
        d[name] = nc.declare_dram_parameter(name, list(shape), dt, isOutput=False).ap()

    din("yq", (C, R))
    din("ykv", (C, T))
    din("xkv", (C, T))
    # weights pre-tiled on host: fm = [n_mt, P, n_kt, 128] (lhsT tiles),
    # rm = [n_cc, P, n_kt, 512] — contiguous per partition line (2KB DMAs).
    din("Wq_t", (FT, P, FT, P))
    din("Wk_t", (FT, P, FT, P))
    din("Wv_t", (2, P, FT, 512))
    din("Wen_k_t", (FT, P, FT, P))
    din("Wen_v_t", (2, P, FT, 512))
    din("Wproj_t", (FT, P, FT, P))
    din("Wq2_t", (FT, P, FT, P))
    din("Wcproj_t", (FT, P, FT, P))
    din("Wd1_t", (DFF // P, P, FT, P))
    din("Wd2_t", (FT, P, DFF // P, P))
    for nm, shape in (("b_attn", (3 * C,)), ("b_proj", (C,)), ("b_en", (2 * C,)),
                      ("b_q", (C,)), ("b_cproj", (C,)), ("b_d1", (DFF,)),
                      ("b_d2", (C,))):
        if nm in flags:
            din(nm, shape)
    for nm in ("ln_w", "ln_b", "ln1_w", "ln1_b", "ln2_w", "ln2_b"):
        if nm in flags:
            din(nm, (C,), dt=F32)
    d["out"] = nc.declare_dram_parameter("out", [C, R], F16, isOutput=True).ap()

    with tile.TileContext(nc) as tc:
        _emit(nc, tc, d, flags)
    nc.compile()
    return nc


def _flags_of(b_attn, b_proj, b_en, b_q, b_cproj, b_d1, b_d2,
              ln_w, ln_b, ln1_w, ln1_b, ln2_w, ln2_b):
    flags = set()
    for nm, arr in (("b_attn", b_attn), ("b_proj", b_proj), ("b_en", b_en),
                    ("b_q", b_q), ("b_cproj", b_cproj), ("b_d1", b_d1),
                    ("b_d2", b_d2)):
        if np.any(np.asarray(arr) != 0):
            flags.add(nm)
    for nm, arr, triv in (("ln_w", ln_w, 1.0), ("ln_b", ln_b, 0.0),
                          ("ln1_w", ln1_w, 1.0), ("ln1_b", ln1_b, 0.0),
                          ("ln2_w", ln2_w, 1.0), ("ln2_b", ln2_b, 0.0)):
        if np.any(np.asarray(arr) != triv):
            flags.add(nm)
    for a, b in (("ln_w", "ln_b"), ("ln1_w", "ln1_b"), ("ln2_w", "ln2_b")):
        if a in flags or b in flags:
            flags.add(a)
            flags.add(b)
    return flags


def prepare_in_maps(x, y, W_attn, b_attn, W_proj, b_proj, ln_w, ln_b,
                    W_en, b_en, W_q, b_q, W_cproj, b_cproj,
                    ln1_w, ln1_b, ln2_w, ln2_b, W_d1, b_d1, W_d2, b_d2,
                    flags):
    f16 = lambda a: np.ascontiguousarray(np.asarray(a, np.float32).astype(np.float16))

    def tile_fm(w):  # [K, M] -> [M//P, P, K//P, P]
        K_, M_ = w.shape
        return np.ascontiguousarray(
            np.asarray(w, np.float32).astype(np.float16)
            .reshape(K_ // P, P, M_ // P, P).transpose(2, 1, 0, 3))

    def tile_rm(w):  # [K, M] -> [M//512, P, K//P, 512]
        K_, M_ = w.shape
        return np.ascontiguousarray(
            np.asarray(w, np.float32).astype(np.float16)
            .reshape(K_ // P, P, M_ // 512, 512).transpose(2, 1, 0, 3))

    W_attn = np.asarray(W_attn)
    W_en = np.asarray(W_en)
    base = {
        "Wq_t": tile_fm(W_attn[:, 0:C]),
        "Wk_t": tile_fm(W_attn[:, C:2 * C]),
        "Wv_t": tile_rm(W_attn[:, 2 * C:3 * C]),
        "Wen_k_t": tile_fm(W_en[:, 0:C]),
        "Wen_v_t": tile_rm(W_en[:, C:2 * C]),
        "Wproj_t": tile_fm(W_proj),
        "Wq2_t": tile_fm(W_q),
        "Wcproj_t": tile_fm(W_cproj),
        "Wd1_t": tile_fm(W_d1),
        "Wd2_t": tile_fm(W_d2),
    }
    opt = {"b_attn": b_attn, "b_proj": b_proj, "b_en": b_en, "b_q": b_q,
           "b_cproj": b_cproj, "b_d1": b_d1, "b_d2": b_d2}
    lnp = {"ln_w": ln_w, "ln_b": ln_b, "ln1_w": ln1_w, "ln1_b": ln1_b,
           "ln2_w": ln2_w, "ln2_b": ln2_b}
    for nm in flags:
        if nm in opt:
            base[nm] = f16(opt[nm])
        else:
            base[nm] = np.ascontiguousarray(lnp[nm], np.float32)

    x = np.asarray(x, np.float32)
    y = np.asarray(y, np.float32)
    yT = [np.ascontiguousarray(y[b].T.astype(np.float16)) for b in range(B)]
    xT = [np.ascontiguousarray(x[b].T.astype(np.float16)) for b in range(B)]
    in_maps = []
    for c in range(NCORES):
        b, h = divmod(c, 2)
        m = dict(base)
        m["ykv"] = yT[b]
        m["xkv"] = xT[b]
        m["yq"] = np.ascontiguousarray(yT[b][:, h * R:(h + 1) * R])
        in_maps.append(m)
    return in_maps


def kernel(x, y, W_attn, b_attn, W_proj, b_proj, ln_w, ln_b,
           W_en, b_en, W_q, b_q, W_cproj, b_cproj,
           ln1_w, ln1_b, ln2_w, ln2_b, W_d1, b_d1, W_d2, b_d2):
    flags = _flags_of(b_attn, b_proj, b_en, b_q, b_cproj, b_d1, b_d2,
                      ln_w, ln_b, ln1_w, ln1_b, ln2_w, ln2_b)
    key = tuple(sorted(flags))
    if key not in _CACHE:
        _CACHE[key] = _build(flags)
    nc = _CACHE[key]

    in_maps = prepare_in_maps(
        x, y, W_attn, b_attn, W_proj, b_proj, ln_w, ln_b,
        W_en, b_en, W_q, b_q, W_cproj, b_cproj,
        ln1_w, ln1_b, ln2_w, ln2_b, W_d1, b_d1, W_d2, b_d2, flags)

    res = run_bass_kernel_spmd(nc, in_maps, list(range(NCORES)))
    out = np.empty((B, T, C), np.float32)
    for c in range(NCORES):
        b, h = divmod(c, 2)
        out[b, h * R:(h + 1) * R, :] = res.results[c]["out"].T.astype(np.float32)
    return out
